# revision 29
# baseline (speedup 1.0000x reference)
"""Trainium2 Bass kernel for nn_CLoss_68521908241007 (retrieval_knn).

Math (per the reference):
  sq_dist[i,j] = ||feat_i||^2 + ||feat2_j||^2 - 2 feat_i . feat2_j
  logits = -temp * sqrt(sq_dist)
  loss = mean_i( logsumexp_j(logits[i,:]) - logits[i, labels_i] )

Sharding: feat rows split across 8 cores (1024 queries each); feat2 replicated.
Each core computes its 1024x8192 block and returns per-row losses; the host
concatenates and takes the mean (the "all-reduce").

Per-core pipeline (PE clock is capped at 1.2 GHz on this system, so PE work
is kept to the bare G matmuls):
  - PE (bf16): psum = G = featT.T @ feat2T      [4x 512-col matmuls per group]
  - DVE: dist_buf = bf16(psum + ybc)            [drains PSUM, adds the y term]
         where ybc = -0.5*(y_sq - 128) broadcast on all 128 partitions
  - ACT: dist = Sqrt(-2*dist_buf + (x_sq+128))  [one 8K-wide call per q-block]
         e    = Exp(-temp*dist)                 [in place, fused row-sum]
  - ACT ops run in two sqrt->exp macro phases; PE/DVE stream the second
    half's matmuls during the first exp phase. Table switches cost ~2.7us,
    so phases are serialized on ACT via a data-chained zero bias (zrow).
  - y_sq broadcast comes out of an all-ones 128x128 reduce matmul (every
    output partition gets the column norms), shifted/scaled by one DVE op.
"""

import numpy as np
from contextlib import ExitStack

import concourse.bass as bass
import concourse.bacc as bacc
import concourse.mybir as mybir
import concourse.tile as tile
from concourse.bass_utils import run_bass_kernel_spmd

AF = mybir.ActivationFunctionType
ALU = mybir.AluOpType
AX = mybir.AxisListType
f32 = mybir.dt.float32
bf16 = mybir.dt.bfloat16

N_CORES = 8
N, M, D = 8192, 8192, 128
NQ = N // N_CORES        # queries per core
QB = NQ // 128           # q-blocks per core (8)
KSEG = 512               # keys per matmul
NKSEG = M // KSEG        # 16
GRP = 4                  # k-segs per psum group (4 banks)
NGRP = NKSEG // GRP      # 4 groups per q-block
HALF = QB // 2           # q-blocks per ACT macro phase


def _body(tc, out_d, featT_d, featn_d, feat2T_d, sel_d, temp_d):
    nc = tc.nc
    with ExitStack() as ctx:
        singles = ctx.enter_context(tc.tile_pool(name="singles", bufs=1))
        sqp = ctx.enter_context(tc.tile_pool(name="sqp", bufs=4))
        distp = ctx.enter_context(tc.tile_pool(name="distp", bufs=QB))
        psp = ctx.enter_context(tc.tile_pool(name="psp", bufs=2, space="PSUM"))
        smallp = ctx.enter_context(tc.tile_pool(name="smallp", bufs=2))

        # ---- inputs -> SBUF; small tensors first so they land early
        featn_sb = singles.tile([128, QB, D], f32)
        nc.sync.dma_start(out=featn_sb,
                          in_=featn_d.rearrange("(b p) d -> p b d", p=128))
        sel_sb = singles.tile([128, QB, D], f32)
        nc.sync.dma_start(out=sel_sb,
                          in_=sel_d.rearrange("(b p) d -> p b d", p=128))
        featT_sb = singles.tile([D, NQ], bf16)
        nc.sync.dma_start(out=featT_sb, in_=featT_d)
        pos_temp = singles.tile([128, 1], f32)
        nc.sync.dma_start(out=pos_temp, in_=temp_d.to_broadcast((128, 1)))
        feat2T_sb = singles.tile([D, M], bf16)
        for c in range(4):
            w = M // 4
            nc.sync.dma_start(out=feat2T_sb[:, c * w:(c + 1) * w],
                              in_=feat2T_d[:, c * w:(c + 1) * w])

        # ---- constants
        ones_mat_f = singles.tile([D, 128], f32)
        nc.vector.memset(ones_mat_f, 1.0)
        ones_mat = singles.tile([D, 128], bf16)
        nc.vector.tensor_copy(ones_mat, ones_mat_f)
        neg_temp = singles.tile([128, 1], f32)
        nc.vector.tensor_scalar_mul(neg_temp, pos_temp, -1.0)

        # ---- ybc[128, M] = bf16(-0.5*(y_sq - 128)) on every partition.
        # The all-ones 128x128 reduce matmul broadcasts the column norms to
        # all output partitions directly in PSUM; one DVE tensor_scalar per
        # 4-bank chunk shifts+scales it into SBUF.
        ybc = singles.tile([128, M], bf16)
        for g in range(NGRP):
            ps_y = psp.tile([128, GRP * KSEG], f32, tag="ps")
            for si in range(GRP):
                s = g * GRP + si
                sq = sqp.tile([128, KSEG], bf16, tag="sq")
                nc.vector.tensor_mul(sq, feat2T_sb[:, s * KSEG:(s + 1) * KSEG],
                                     feat2T_sb[:, s * KSEG:(s + 1) * KSEG])
                nc.tensor.matmul(ps_y[:, si * KSEG:(si + 1) * KSEG],
                                 lhsT=ones_mat, rhs=sq, start=True, stop=True)
            nc.vector.tensor_scalar(
                out=ybc[:, g * GRP * KSEG:(g + 1) * GRP * KSEG],
                in0=ps_y, scalar1=-128.0, scalar2=-0.5,
                op0=ALU.add, op1=ALU.mult)

        # ---- x_sq (+128 shift) for the sqrt bias -- DVE
        x_sq = singles.tile([128, QB], f32)
        for b in range(QB):
            fsq = smallp.tile([128, D], f32, tag="fsq")
            nc.vector.tensor_mul(fsq, featn_sb[:, b, :], featn_sb[:, b, :])
            nc.vector.reduce_sum(x_sq[:, b:b + 1], fsq, axis=AX.X)
        xb = singles.tile([128, QB], f32)
        nc.vector.tensor_scalar_add(xb, x_sq, 128.0)

        # ---- all matmuls + PSUM drains (PE/DVE stream independent of ACT)
        dist_tiles = []
        for b in range(QB):
            dist_t = distp.tile([128, M], bf16, tag="dist")
            dist_tiles.append(dist_t)
            for g in range(NGRP):
                ps = psp.tile([128, GRP * KSEG], f32, tag="ps")
                for si in range(GRP):
                    nc.tensor.matmul(
                        ps[:, si * KSEG:(si + 1) * KSEG],
                        lhsT=featT_sb[:, b * 128:(b + 1) * 128],
                        rhs=feat2T_sb[:, (g * GRP + si) * KSEG:
                                      (g * GRP + si + 1) * KSEG],
                        start=True, stop=True)
                nc.vector.tensor_add(
                    dist_t[:, g * GRP * KSEG:(g + 1) * GRP * KSEG],
                    ps, ybc[:, g * GRP * KSEG:(g + 1) * GRP * KSEG])

        # ---- picked-label squared distance (DVE)
        psq = singles.tile([128, QB], f32)
        diff_all = singles.tile([128, QB, D], f32)
        nc.vector.tensor_sub(diff_all, featn_sb, sel_sb)
        for b in range(QB):
            dsq = smallp.tile([128, D], f32, tag="fsq")
            nc.vector.tensor_mul(dsq, diff_all[:, b, :], diff_all[:, b, :])
            nc.vector.reduce_sum(psq[:, b:b + 1], dsq, axis=AX.X)

        # ---- ACT phases: pairs of [sqrt x2, exp x2], chained via zero-bias
        # rows so the scheduler cannot interleave sqrt/exp table sets within
        # a pair; pairs overlap ACT exp time with DVE drains of later qbs.
        NPH = 4
        PER = QB // NPH
        S = singles.tile([128, QB], f32)
        pdist = singles.tile([128, QB], f32)
        zrows = singles.tile([128, NPH], f32)
        m2s = singles.tile([128, NPH], f32)
        for h in range(NPH):
            qbs = range(h * PER, (h + 1) * PER)
            if h == 0:
                scale_h = -2.0
            else:
                # -2.0 derived from the previous pair's exp accumulator:
                # forces sqrt(pair h) after exp(pair h-1) on ACT (no table
                # thrash in either direction)
                nc.vector.tensor_scalar(
                    out=m2s[:, h:h + 1], in0=S[:, h * PER - 1:h * PER],
                    scalar1=0.0, scalar2=-2.0, op0=ALU.mult, op1=ALU.add)
                scale_h = m2s[:, h:h + 1]
            for b in qbs:
                nc.scalar.activation(
                    out=dist_tiles[b], in_=dist_tiles[b], func=AF.Sqrt,
                    bias=xb[:, b:b + 1], scale=scale_h)
            if h == NPH - 1:
                # picked-label distance; still inside a sqrt-table window
                nc.scalar.activation(out=pdist, in_=psq, func=AF.Sqrt,
                                     bias=0.0, scale=1.0)
                nc.vector.tensor_scalar_mul(zrows[:, h:h + 1],
                                            pdist[:, 0:1], 0.0)
            else:
                last = (h + 1) * PER - 1
                nc.vector.tensor_scalar_mul(zrows[:, h:h + 1],
                                            dist_tiles[last][:, M - 1:M], 0.0)
            for b in qbs:
                nc.scalar.activation(
                    out=dist_tiles[b], in_=dist_tiles[b], func=AF.Exp,
                    bias=zrows[:, h:h + 1], scale=neg_temp[:, 0:1],
                    accum_out=S[:, b:b + 1])

        # ---- finals: loss_row = Ln(S) + temp * pdist
        logz = singles.tile([128, QB], f32)
        nc.scalar.activation(out=logz, in_=S, func=AF.Ln, bias=0.0, scale=1.0)
        picked = singles.tile([128, QB], f32)
        nc.vector.tensor_scalar_mul(picked, pdist, pos_temp[:, 0:1])
        loss_t = singles.tile([128, QB], f32)
        nc.vector.tensor_add(loss_t, picked, logz)
        nc.sync.dma_start(out=out_d, in_=loss_t)


def build_program():
    nc = bacc.Bacc("TRN2", target_bir_lowering=False, debug=False,
                   num_devices=N_CORES)
    featT = nc.dram_tensor("featT", [D, NQ], bf16, kind="ExternalInput").ap()
    featn = nc.dram_tensor("featn", [NQ, D], f32, kind="ExternalInput").ap()
    feat2T = nc.dram_tensor("feat2T", [D, M], bf16, kind="ExternalInput").ap()
    sel = nc.dram_tensor("sel", [NQ, D], f32, kind="ExternalInput").ap()
    temp = nc.dram_tensor("temp", [1, 1], f32, kind="ExternalInput").ap()
    out = nc.dram_tensor("out", [128, QB], f32, kind="ExternalOutput").ap()
    with tile.TileContext(nc) as tc:
        _body(tc, out, featT, featn, feat2T, sel, temp)
    nc.compile()
    return nc


def make_in_maps(feat, feat2, temp, labels):
    import ml_dtypes
    feat = np.ascontiguousarray(np.asarray(feat, dtype=np.float32))
    feat2 = np.ascontiguousarray(np.asarray(feat2, dtype=np.float32))
    labels_np = np.asarray(labels).astype(np.int64)
    temp_np = np.asarray(temp, dtype=np.float32).reshape(1, 1)
    feat2T = np.ascontiguousarray(feat2.T).astype(ml_dtypes.bfloat16)
    sel_full = feat2[labels_np]
    in_maps = []
    for c in range(N_CORES):
        fs = feat[c * NQ:(c + 1) * NQ]
        in_maps.append({
            "featT": np.ascontiguousarray(fs.T).astype(ml_dtypes.bfloat16),
            "featn": fs,
            "feat2T": feat2T,
            "sel": np.ascontiguousarray(sel_full[c * NQ:(c + 1) * NQ]),
            "temp": temp_np,
        })
    return in_maps


def combine_outputs(per_core_outs):
    # out[p, b] is the loss for query q = b*128 + p of that core's shard
    rows = [np.asarray(o).T.reshape(-1) for o in per_core_outs]
    return np.float32(np.concatenate(rows).mean())


_PROGRAM = None


def kernel(feat, feat2, temp, labels):
    global _PROGRAM
    if _PROGRAM is None:
        _PROGRAM = build_program()
    in_maps = make_in_maps(feat, feat2, temp, labels)
    res = run_bass_kernel_spmd(_PROGRAM, in_maps, core_ids=list(range(N_CORES)))
    return combine_outputs([r["out"] for r in res.results])


# revision 30
# speedup vs baseline: 1.2402x; 1.2402x over previous
"""Trainium2 Bass kernel for nn_CLoss_68521908241007 (retrieval_knn).

Math (per the reference):
  sq_dist[i,j] = ||feat_i||^2 + ||feat2_j||^2 - 2 feat_i . feat2_j
  logits = -temp * sqrt(sq_dist)
  loss = mean_i( logsumexp_j(logits[i,:]) - logits[i, labels_i] )

Sharding: feat rows split across 8 cores (1024 queries each); feat2 replicated.
Each core computes its 1024x8192 block and returns per-row losses; the host
concatenates and takes the mean (the "all-reduce").

Per-core pipeline (PE clock is capped at 1.2 GHz on this system, so PE work
is kept to the bare G matmuls):
  - PE (bf16): psum = G = featT.T @ feat2T      [4x 512-col matmuls per group]
  - DVE: dist_buf = bf16(psum + ybc)            [drains PSUM, adds the y term]
         where ybc = -0.5*(y_sq - 128) broadcast on all 128 partitions
  - ACT: dist = Sqrt(-2*dist_buf + (x_sq+128))  [one 8K-wide call per q-block]
         e    = Exp(-temp*dist)                 [in place, fused row-sum]
  - ACT ops run in two sqrt->exp macro phases; PE/DVE stream the second
    half's matmuls during the first exp phase. Table switches cost ~2.7us,
    so phases are serialized on ACT via a data-chained zero bias (zrow).
  - y_sq broadcast comes out of an all-ones 128x128 reduce matmul (every
    output partition gets the column norms), shifted/scaled by one DVE op.
"""

import numpy as np
from contextlib import ExitStack

import concourse.bass as bass
import concourse.bacc as bacc
import concourse.mybir as mybir
import concourse.tile as tile
from concourse.bass_utils import run_bass_kernel_spmd

AF = mybir.ActivationFunctionType
ALU = mybir.AluOpType
AX = mybir.AxisListType
f32 = mybir.dt.float32
bf16 = mybir.dt.bfloat16

N_CORES = 8
N, M, D = 8192, 8192, 128
NQ = N // N_CORES        # queries per core
QB = NQ // 128           # q-blocks per core (8)
KSEG = 512               # keys per matmul
NKSEG = M // KSEG        # 16
GRP = 4                  # k-segs per psum group (4 banks)
NGRP = NKSEG // GRP      # 4 groups per q-block
HALF = QB // 2           # q-blocks per ACT macro phase


def _body(tc, out_d, featT_d, featn_d, feat2T_d, sel_d, temp_d):
    nc = tc.nc
    with ExitStack() as ctx:
        singles = ctx.enter_context(tc.tile_pool(name="singles", bufs=1))
        sqp = ctx.enter_context(tc.tile_pool(name="sqp", bufs=4))
        distp = ctx.enter_context(tc.tile_pool(name="distp", bufs=QB))
        psp = ctx.enter_context(tc.tile_pool(name="psp", bufs=2, space="PSUM"))
        smallp = ctx.enter_context(tc.tile_pool(name="smallp", bufs=2))

        # ---- inputs -> SBUF; small tensors first so they land early
        featn_sb = singles.tile([128, QB, D], f32)
        nc.sync.dma_start(out=featn_sb,
                          in_=featn_d.rearrange("(b p) d -> p b d", p=128))
        sel_sb = singles.tile([128, QB, D], f32)
        nc.sync.dma_start(out=sel_sb,
                          in_=sel_d.rearrange("(b p) d -> p b d", p=128))
        featT_sb = singles.tile([D, NQ], bf16)
        nc.sync.dma_start(out=featT_sb, in_=featT_d)
        pos_temp = singles.tile([128, 1], f32)
        nc.sync.dma_start(out=pos_temp, in_=temp_d.to_broadcast((128, 1)))
        feat2T_sb = singles.tile([D, M], bf16)
        for c in range(4):
            w = M // 4
            nc.sync.dma_start(out=feat2T_sb[:, c * w:(c + 1) * w],
                              in_=feat2T_d[:, c * w:(c + 1) * w])

        # ---- constants
        ones_mat_f = singles.tile([D, 128], f32)
        nc.vector.memset(ones_mat_f, 1.0)
        ones_mat = singles.tile([D, 128], bf16)
        nc.vector.tensor_copy(ones_mat, ones_mat_f)
        neg_temp = singles.tile([128, 1], f32)
        nc.vector.tensor_scalar_mul(neg_temp, pos_temp, -1.0)

        # ---- ybc[128, M] = bf16(-0.5*(y_sq - 128)) on every partition.
        # The all-ones 128x128 reduce matmul broadcasts the column norms to
        # all output partitions directly in PSUM; one DVE tensor_scalar per
        # 4-bank chunk shifts+scales it into SBUF.
        ybc = singles.tile([128, M], bf16)
        for g in range(NGRP):
            ps_y = psp.tile([128, GRP * KSEG], f32, tag="ps")
            for si in range(GRP):
                s = g * GRP + si
                sq = sqp.tile([128, KSEG], bf16, tag="sq")
                nc.vector.tensor_mul(sq, feat2T_sb[:, s * KSEG:(s + 1) * KSEG],
                                     feat2T_sb[:, s * KSEG:(s + 1) * KSEG])
                nc.tensor.matmul(ps_y[:, si * KSEG:(si + 1) * KSEG],
                                 lhsT=ones_mat, rhs=sq, start=True, stop=True)
            nc.vector.tensor_scalar(
                out=ybc[:, g * GRP * KSEG:(g + 1) * GRP * KSEG],
                in0=ps_y, scalar1=-128.0, scalar2=-0.5,
                op0=ALU.add, op1=ALU.mult)

        # ---- x_sq (+128 shift) for the sqrt bias -- DVE
        x_sq = singles.tile([128, QB], f32)
        for b in range(QB):
            fsq = smallp.tile([128, D], f32, tag="fsq")
            nc.vector.tensor_mul(fsq, featn_sb[:, b, :], featn_sb[:, b, :])
            nc.vector.reduce_sum(x_sq[:, b:b + 1], fsq, axis=AX.X)
        xb = singles.tile([128, QB], f32)
        nc.vector.tensor_scalar_add(xb, x_sq, 128.0)

        # ---- picked-label squared distance (DVE, early: ACT needs pdist in
        # the last sqrt-table window)
        psq = singles.tile([128, QB], f32)
        diff_all = singles.tile([128, QB, D], f32)
        nc.vector.tensor_sub(diff_all, featn_sb, sel_sb)
        for b in range(QB):
            dsq = smallp.tile([128, D], f32, tag="fsq")
            nc.vector.tensor_mul(dsq, diff_all[:, b, :], diff_all[:, b, :])
            nc.vector.reduce_sum(psq[:, b:b + 1], dsq, axis=AX.X)

        # ---- main pipeline, emitted in pair-of-qb chunks so every engine's
        # priority queue interleaves: [mains+drains x2qb][sqrt x2][zrow]
        # [exp x2] ... ACT table phases are data-chained in both directions
        # (zrow: exp after last sqrt of the pair; m2: sqrt of pair h after
        # exp of pair h-1) so the ~2.7us table reloads stay at 2 per pair.
        NPH = 4
        PER = QB // NPH
        S = singles.tile([128, QB], f32)
        pdist = singles.tile([128, QB], f32)
        zrows = singles.tile([128, NPH], f32)
        m2s = singles.tile([128, NPH], f32)
        dist_tiles = []
        for h in range(NPH):
            qbs = range(h * PER, (h + 1) * PER)
            for b in qbs:
                dist_t = distp.tile([128, M], bf16, tag="dist")
                dist_tiles.append(dist_t)
                for g in range(NGRP):
                    ps = psp.tile([128, GRP * KSEG], f32, tag="ps")
                    for si in range(GRP):
                        nc.tensor.matmul(
                            ps[:, si * KSEG:(si + 1) * KSEG],
                            lhsT=featT_sb[:, b * 128:(b + 1) * 128],
                            rhs=feat2T_sb[:, (g * GRP + si) * KSEG:
                                          (g * GRP + si + 1) * KSEG],
                            start=True, stop=True)
                    nc.vector.tensor_add(
                        dist_t[:, g * GRP * KSEG:(g + 1) * GRP * KSEG],
                        ps, ybc[:, g * GRP * KSEG:(g + 1) * GRP * KSEG])
            if h == 0:
                scale_h = -2.0
            else:
                nc.vector.tensor_scalar(
                    out=m2s[:, h:h + 1], in0=S[:, h * PER - 1:h * PER],
                    scalar1=0.0, scalar2=-2.0, op0=ALU.mult, op1=ALU.add)
                scale_h = m2s[:, h:h + 1]
            for b in qbs:
                nc.scalar.activation(
                    out=dist_tiles[b], in_=dist_tiles[b], func=AF.Sqrt,
                    bias=xb[:, b:b + 1], scale=scale_h)
            if h == NPH - 1:
                # picked-label distance; still inside a sqrt-table window
                nc.scalar.activation(out=pdist, in_=psq, func=AF.Sqrt,
                                     bias=0.0, scale=1.0)
                nc.vector.tensor_scalar_mul(zrows[:, h:h + 1],
                                            pdist[:, 0:1], 0.0)
            else:
                last = (h + 1) * PER - 1
                nc.vector.tensor_scalar_mul(zrows[:, h:h + 1],
                                            dist_tiles[last][:, M - 1:M], 0.0)
            for b in qbs:
                nc.scalar.activation(
                    out=dist_tiles[b], in_=dist_tiles[b], func=AF.Exp,
                    bias=zrows[:, h:h + 1], scale=neg_temp[:, 0:1],
                    accum_out=S[:, b:b + 1])

        # ---- finals: loss_row = Ln(S) + temp * pdist
        logz = singles.tile([128, QB], f32)
        nc.scalar.activation(out=logz, in_=S, func=AF.Ln, bias=0.0, scale=1.0)
        picked = singles.tile([128, QB], f32)
        nc.vector.tensor_scalar_mul(picked, pdist, pos_temp[:, 0:1])
        loss_t = singles.tile([128, QB], f32)
        nc.vector.tensor_add(loss_t, picked, logz)
        nc.sync.dma_start(out=out_d, in_=loss_t)


def build_program():
    nc = bacc.Bacc("TRN2", target_bir_lowering=False, debug=False,
                   num_devices=N_CORES)
    featT = nc.dram_tensor("featT", [D, NQ], bf16, kind="ExternalInput").ap()
    featn = nc.dram_tensor("featn", [NQ, D], f32, kind="ExternalInput").ap()
    feat2T = nc.dram_tensor("feat2T", [D, M], bf16, kind="ExternalInput").ap()
    sel = nc.dram_tensor("sel", [NQ, D], f32, kind="ExternalInput").ap()
    temp = nc.dram_tensor("temp", [1, 1], f32, kind="ExternalInput").ap()
    out = nc.dram_tensor("out", [128, QB], f32, kind="ExternalOutput").ap()
    with tile.TileContext(nc) as tc:
        _body(tc, out, featT, featn, feat2T, sel, temp)
    nc.compile()
    return nc


def make_in_maps(feat, feat2, temp, labels):
    import ml_dtypes
    feat = np.ascontiguousarray(np.asarray(feat, dtype=np.float32))
    feat2 = np.ascontiguousarray(np.asarray(feat2, dtype=np.float32))
    labels_np = np.asarray(labels).astype(np.int64)
    temp_np = np.asarray(temp, dtype=np.float32).reshape(1, 1)
    feat2T = np.ascontiguousarray(feat2.T).astype(ml_dtypes.bfloat16)
    sel_full = feat2[labels_np]
    in_maps = []
    for c in range(N_CORES):
        fs = feat[c * NQ:(c + 1) * NQ]
        in_maps.append({
            "featT": np.ascontiguousarray(fs.T).astype(ml_dtypes.bfloat16),
            "featn": fs,
            "feat2T": feat2T,
            "sel": np.ascontiguousarray(sel_full[c * NQ:(c + 1) * NQ]),
            "temp": temp_np,
        })
    return in_maps


def combine_outputs(per_core_outs):
    # out[p, b] is the loss for query q = b*128 + p of that core's shard
    rows = [np.asarray(o).T.reshape(-1) for o in per_core_outs]
    return np.float32(np.concatenate(rows).mean())


_PROGRAM = None


def kernel(feat, feat2, temp, labels):
    global _PROGRAM
    if _PROGRAM is None:
        _PROGRAM = build_program()
    in_maps = make_in_maps(feat, feat2, temp, labels)
    res = run_bass_kernel_spmd(_PROGRAM, in_maps, core_ids=list(range(N_CORES)))
    return combine_outputs([r["out"] for r in res.results])


# revision 31
# speedup vs baseline: 1.2571x; 1.0137x over previous
"""Trainium2 Bass kernel for nn_CLoss_68521908241007 (retrieval_knn).

Math (per the reference):
  sq_dist[i,j] = ||feat_i||^2 + ||feat2_j||^2 - 2 feat_i . feat2_j
  logits = -temp * sqrt(sq_dist)
  loss = mean_i( logsumexp_j(logits[i,:]) - logits[i, labels_i] )

Sharding: feat rows split across 8 cores (1024 queries each); feat2 replicated.
Each core computes its 1024x8192 block and returns per-row losses; the host
concatenates and takes the mean (the "all-reduce").

Per-core pipeline (PE clock is capped at 1.2 GHz on this system, so PE work
is kept to the bare G matmuls):
  - PE (bf16): psum = G = featT.T @ feat2T      [4x 512-col matmuls per group]
  - DVE: dist_buf = bf16(psum + ybc)            [drains PSUM, adds the y term]
         where ybc = -0.5*(y_sq - 128) broadcast on all 128 partitions
  - ACT: dist = Sqrt(-2*dist_buf + (x_sq+128))  [one 8K-wide call per q-block]
         e    = Exp(-temp*dist)                 [in place, fused row-sum]
  - ACT ops run in two sqrt->exp macro phases; PE/DVE stream the second
    half's matmuls during the first exp phase. Table switches cost ~2.7us,
    so phases are serialized on ACT via a data-chained zero bias (zrow).
  - y_sq broadcast comes out of an all-ones 128x128 reduce matmul (every
    output partition gets the column norms), shifted/scaled by one DVE op.
"""

import numpy as np
from contextlib import ExitStack

import concourse.bass as bass
import concourse.bacc as bacc
import concourse.mybir as mybir
import concourse.tile as tile
from concourse.bass_utils import run_bass_kernel_spmd

AF = mybir.ActivationFunctionType
ALU = mybir.AluOpType
AX = mybir.AxisListType
f32 = mybir.dt.float32
bf16 = mybir.dt.bfloat16

N_CORES = 8
N, M, D = 8192, 8192, 128
NQ = N // N_CORES        # queries per core
QB = NQ // 128           # q-blocks per core (8)
KSEG = 512               # keys per matmul
NKSEG = M // KSEG        # 16
GRP = 4                  # k-segs per psum group (4 banks)
NGRP = NKSEG // GRP      # 4 groups per q-block
HALF = QB // 2           # q-blocks per ACT macro phase


def _body(tc, out_d, featT_d, featn_d, feat2T_d, sel_d, temp_d):
    nc = tc.nc
    with ExitStack() as ctx:
        singles = ctx.enter_context(tc.tile_pool(name="singles", bufs=1))
        sqp = ctx.enter_context(tc.tile_pool(name="sqp", bufs=4))
        distp = ctx.enter_context(tc.tile_pool(name="distp", bufs=QB))
        psp = ctx.enter_context(tc.tile_pool(name="psp", bufs=2, space="PSUM"))
        smallp = ctx.enter_context(tc.tile_pool(name="smallp", bufs=2))

        # ---- inputs -> SBUF; small tensors first so they land early
        featn_sb = singles.tile([128, QB, D], f32)
        nc.sync.dma_start(out=featn_sb,
                          in_=featn_d.rearrange("(b p) d -> p b d", p=128))
        sel_sb = singles.tile([128, QB, D], f32)
        nc.sync.dma_start(out=sel_sb,
                          in_=sel_d.rearrange("(b p) d -> p b d", p=128))
        featT_sb = singles.tile([D, NQ], bf16)
        nc.sync.dma_start(out=featT_sb, in_=featT_d)
        pos_temp = singles.tile([128, 1], f32)
        nc.sync.dma_start(out=pos_temp, in_=temp_d.to_broadcast((128, 1)))
        feat2T_sb = singles.tile([D, M], bf16)
        for c in range(4):
            w = M // 4
            nc.sync.dma_start(out=feat2T_sb[:, c * w:(c + 1) * w],
                              in_=feat2T_d[:, c * w:(c + 1) * w])

        # ---- constants
        ones_mat_f = singles.tile([D, 128], f32)
        nc.vector.memset(ones_mat_f, 1.0)
        ones_mat = singles.tile([D, 128], bf16)
        nc.vector.tensor_copy(ones_mat, ones_mat_f)
        neg_temp = singles.tile([128, 1], f32)
        nc.vector.tensor_scalar_mul(neg_temp, pos_temp, -1.0)

        # ---- ybc[128, M] = bf16(-0.5*(y_sq - 128)) on every partition.
        # The all-ones 128x128 reduce matmul broadcasts the column norms to
        # all output partitions directly in PSUM; one DVE tensor_scalar per
        # 4-bank chunk shifts+scales it into SBUF.
        # Squares run on ACT (Square is in every activation-table set, and
        # ACT is otherwise idle until the first sqrt) so DVE's pre-qb0
        # critical chain is just the ybc shifts + qb0 drains.
        ybc = singles.tile([128, M], bf16)
        for g in range(NGRP):
            ps_y = psp.tile([128, GRP * KSEG], f32, tag="ps")
            for si in range(GRP):
                s = g * GRP + si
                sq = sqp.tile([128, KSEG], bf16, tag="sq")
                nc.scalar.activation(
                    out=sq, in_=feat2T_sb[:, s * KSEG:(s + 1) * KSEG],
                    func=AF.Square, bias=0.0, scale=1.0)
                nc.tensor.matmul(ps_y[:, si * KSEG:(si + 1) * KSEG],
                                 lhsT=ones_mat, rhs=sq, start=True, stop=True)
            nc.vector.tensor_scalar(
                out=ybc[:, g * GRP * KSEG:(g + 1) * GRP * KSEG],
                in0=ps_y, scalar1=-128.0, scalar2=-0.5,
                op0=ALU.add, op1=ALU.mult)

        # ---- x_sq (+128 shift) for the sqrt bias -- ACT Square with fused
        # row-sum (also in ACT's idle startup window)
        x_sq = singles.tile([128, QB], f32)
        for b in range(QB):
            fsq = smallp.tile([128, D], f32, tag="fsq")
            nc.scalar.activation(out=fsq, in_=featn_sb[:, b, :],
                                 func=AF.Square, bias=0.0, scale=1.0,
                                 accum_out=x_sq[:, b:b + 1])
        xb = singles.tile([128, QB], f32)
        nc.vector.tensor_scalar_add(xb, x_sq, 128.0)

        # ---- picked-label squared distance (DVE, early: ACT needs pdist in
        # the last sqrt-table window)
        psq = singles.tile([128, QB], f32)
        diff_all = singles.tile([128, QB, D], f32)
        nc.vector.tensor_sub(diff_all, featn_sb, sel_sb)
        for b in range(QB):
            dsq = smallp.tile([128, D], f32, tag="fsq")
            nc.vector.tensor_mul(dsq, diff_all[:, b, :], diff_all[:, b, :])
            nc.vector.reduce_sum(psq[:, b:b + 1], dsq, axis=AX.X)

        # ---- main pipeline, emitted in pair-of-qb chunks so every engine's
        # priority queue interleaves: [mains+drains x2qb][sqrt x2][zrow]
        # [exp x2] ... ACT table phases are data-chained in both directions
        # (zrow: exp after last sqrt of the pair; m2: sqrt of pair h after
        # exp of pair h-1) so the ~2.7us table reloads stay at 2 per pair.
        NPH = 4
        PER = QB // NPH
        S = singles.tile([128, QB], f32)
        pdist = singles.tile([128, QB], f32)
        zrows = singles.tile([128, NPH], f32)
        m2s = singles.tile([128, NPH], f32)
        dist_tiles = []
        for h in range(NPH):
            qbs = range(h * PER, (h + 1) * PER)
            for b in qbs:
                dist_t = distp.tile([128, M], bf16, tag="dist")
                dist_tiles.append(dist_t)
                for g in range(NGRP):
                    ps = psp.tile([128, GRP * KSEG], f32, tag="ps")
                    for si in range(GRP):
                        nc.tensor.matmul(
                            ps[:, si * KSEG:(si + 1) * KSEG],
                            lhsT=featT_sb[:, b * 128:(b + 1) * 128],
                            rhs=feat2T_sb[:, (g * GRP + si) * KSEG:
                                          (g * GRP + si + 1) * KSEG],
                            start=True, stop=True)
                    nc.vector.tensor_add(
                        dist_t[:, g * GRP * KSEG:(g + 1) * GRP * KSEG],
                        ps, ybc[:, g * GRP * KSEG:(g + 1) * GRP * KSEG])
            if h == 0:
                scale_h = -2.0
            else:
                nc.vector.tensor_scalar(
                    out=m2s[:, h:h + 1], in0=S[:, h * PER - 1:h * PER],
                    scalar1=0.0, scalar2=-2.0, op0=ALU.mult, op1=ALU.add)
                scale_h = m2s[:, h:h + 1]
            for b in qbs:
                nc.scalar.activation(
                    out=dist_tiles[b], in_=dist_tiles[b], func=AF.Sqrt,
                    bias=xb[:, b:b + 1], scale=scale_h)
            if h == NPH - 1:
                # picked-label distance; still inside a sqrt-table window
                nc.scalar.activation(out=pdist, in_=psq, func=AF.Sqrt,
                                     bias=0.0, scale=1.0)
                nc.vector.tensor_scalar_mul(zrows[:, h:h + 1],
                                            pdist[:, 0:1], 0.0)
            else:
                last = (h + 1) * PER - 1
                nc.vector.tensor_scalar_mul(zrows[:, h:h + 1],
                                            dist_tiles[last][:, M - 1:M], 0.0)
            for b in qbs:
                nc.scalar.activation(
                    out=dist_tiles[b], in_=dist_tiles[b], func=AF.Exp,
                    bias=zrows[:, h:h + 1], scale=neg_temp[:, 0:1],
                    accum_out=S[:, b:b + 1])

        # ---- finals: loss_row = Ln(S) + temp * pdist
        logz = singles.tile([128, QB], f32)
        nc.scalar.activation(out=logz, in_=S, func=AF.Ln, bias=0.0, scale=1.0)
        picked = singles.tile([128, QB], f32)
        nc.vector.tensor_scalar_mul(picked, pdist, pos_temp[:, 0:1])
        loss_t = singles.tile([128, QB], f32)
        nc.vector.tensor_add(loss_t, picked, logz)
        nc.sync.dma_start(out=out_d, in_=loss_t)


def build_program():
    nc = bacc.Bacc("TRN2", target_bir_lowering=False, debug=False,
                   num_devices=N_CORES)
    featT = nc.dram_tensor("featT", [D, NQ], bf16, kind="ExternalInput").ap()
    featn = nc.dram_tensor("featn", [NQ, D], f32, kind="ExternalInput").ap()
    feat2T = nc.dram_tensor("feat2T", [D, M], bf16, kind="ExternalInput").ap()
    sel = nc.dram_tensor("sel", [NQ, D], f32, kind="ExternalInput").ap()
    temp = nc.dram_tensor("temp", [1, 1], f32, kind="ExternalInput").ap()
    out = nc.dram_tensor("out", [128, QB], f32, kind="ExternalOutput").ap()
    with tile.TileContext(nc) as tc:
        _body(tc, out, featT, featn, feat2T, sel, temp)
    nc.compile()
    return nc


def make_in_maps(feat, feat2, temp, labels):
    import ml_dtypes
    feat = np.ascontiguousarray(np.asarray(feat, dtype=np.float32))
    feat2 = np.ascontiguousarray(np.asarray(feat2, dtype=np.float32))
    labels_np = np.asarray(labels).astype(np.int64)
    temp_np = np.asarray(temp, dtype=np.float32).reshape(1, 1)
    feat2T = np.ascontiguousarray(feat2.T).astype(ml_dtypes.bfloat16)
    sel_full = feat2[labels_np]
    in_maps = []
    for c in range(N_CORES):
        fs = feat[c * NQ:(c + 1) * NQ]
        in_maps.append({
            "featT": np.ascontiguousarray(fs.T).astype(ml_dtypes.bfloat16),
            "featn": fs,
            "feat2T": feat2T,
            "sel": np.ascontiguousarray(sel_full[c * NQ:(c + 1) * NQ]),
            "temp": temp_np,
        })
    return in_maps


def combine_outputs(per_core_outs):
    # out[p, b] is the loss for query q = b*128 + p of that core's shard
    rows = [np.asarray(o).T.reshape(-1) for o in per_core_outs]
    return np.float32(np.concatenate(rows).mean())


_PROGRAM = None


def kernel(feat, feat2, temp, labels):
    global _PROGRAM
    if _PROGRAM is None:
        _PROGRAM = build_program()
    in_maps = make_in_maps(feat, feat2, temp, labels)
    res = run_bass_kernel_spmd(_PROGRAM, in_maps, core_ids=list(range(N_CORES)))
    return combine_outputs([r["out"] for r in res.results])


# revision 32
# speedup vs baseline: 1.2799x; 1.0181x over previous
"""Trainium2 Bass kernel for nn_CLoss_68521908241007 (retrieval_knn).

Math (per the reference):
  sq_dist[i,j] = ||feat_i||^2 + ||feat2_j||^2 - 2 feat_i . feat2_j
  logits = -temp * sqrt(sq_dist)
  loss = mean_i( logsumexp_j(logits[i,:]) - logits[i, labels_i] )

Sharding: feat rows split across 8 cores (1024 queries each); feat2 replicated.
Each core computes its 1024x8192 block and returns per-row losses; the host
concatenates and takes the mean (the "all-reduce").

Per-core pipeline (PE clock is capped at 1.2 GHz on this system, so PE work
is kept to the bare G matmuls):
  - PE (bf16): psum = G = featT.T @ feat2T      [4x 512-col matmuls per group]
  - DVE: dist_buf = bf16(psum + ybc)            [drains PSUM, adds the y term]
         where ybc = -0.5*(y_sq - 128) broadcast on all 128 partitions
  - ACT: dist = Sqrt(-2*dist_buf + (x_sq+128))  [one 8K-wide call per q-block]
         e    = Exp(-temp*dist)                 [in place, fused row-sum]
  - ACT ops run in two sqrt->exp macro phases; PE/DVE stream the second
    half's matmuls during the first exp phase. Table switches cost ~2.7us,
    so phases are serialized on ACT via a data-chained zero bias (zrow).
  - y_sq broadcast comes out of an all-ones 128x128 reduce matmul (every
    output partition gets the column norms), shifted/scaled by one DVE op.
"""

import numpy as np
from contextlib import ExitStack

import concourse.bass as bass
import concourse.bacc as bacc
import concourse.mybir as mybir
import concourse.tile as tile
from concourse.bass_utils import run_bass_kernel_spmd

AF = mybir.ActivationFunctionType
ALU = mybir.AluOpType
AX = mybir.AxisListType
f32 = mybir.dt.float32
bf16 = mybir.dt.bfloat16

N_CORES = 8
N, M, D = 8192, 8192, 128
NQ = N // N_CORES        # queries per core
QB = NQ // 128           # q-blocks per core (8)
KSEG = 512               # keys per matmul
NKSEG = M // KSEG        # 16
GRP = 4                  # k-segs per psum group (4 banks)
NGRP = NKSEG // GRP      # 4 groups per q-block
HALF = QB // 2           # q-blocks per ACT macro phase


def _body(tc, out_d, featT_d, featn_d, feat2T_d, sel_d, temp_d):
    nc = tc.nc
    with ExitStack() as ctx:
        singles = ctx.enter_context(tc.tile_pool(name="singles", bufs=1))
        sqp = ctx.enter_context(tc.tile_pool(name="sqp", bufs=4))
        distp = ctx.enter_context(tc.tile_pool(name="distp", bufs=QB))
        psp = ctx.enter_context(tc.tile_pool(name="psp", bufs=2, space="PSUM"))
        smallp = ctx.enter_context(tc.tile_pool(name="smallp", bufs=2))

        # ---- inputs -> SBUF; feat2T first: it heads the critical y_sq chain
        feat2T_sb = singles.tile([D, M], bf16)
        for c in range(4):
            w = M // 4
            nc.sync.dma_start(out=feat2T_sb[:, c * w:(c + 1) * w],
                              in_=feat2T_d[:, c * w:(c + 1) * w])
        featT_sb = singles.tile([D, NQ], bf16)
        nc.sync.dma_start(out=featT_sb, in_=featT_d)
        featn_sb = singles.tile([128, QB, D], bf16)
        nc.sync.dma_start(out=featn_sb,
                          in_=featn_d.rearrange("(b p) d -> p b d", p=128))
        sel_sb = singles.tile([128, QB, D], bf16)
        nc.sync.dma_start(out=sel_sb,
                          in_=sel_d.rearrange("(b p) d -> p b d", p=128))
        pos_temp = singles.tile([128, 1], f32)
        nc.sync.dma_start(out=pos_temp, in_=temp_d.to_broadcast((128, 1)))

        # ---- constants
        ones_mat_f = singles.tile([D, 128], f32)
        nc.vector.memset(ones_mat_f, 1.0)
        ones_mat = singles.tile([D, 128], bf16)
        nc.vector.tensor_copy(ones_mat, ones_mat_f)
        neg_temp = singles.tile([128, 1], f32)
        nc.vector.tensor_scalar_mul(neg_temp, pos_temp, -1.0)

        # ---- ybc[128, M] = bf16(-0.5*(y_sq - 128)) on every partition.
        # The all-ones 128x128 reduce matmul broadcasts the column norms to
        # all output partitions directly in PSUM; one DVE tensor_scalar per
        # 4-bank chunk shifts+scales it into SBUF.
        # Squares run on ACT (Square is in every activation-table set, and
        # ACT is otherwise idle until the first sqrt) so DVE's pre-qb0
        # critical chain is just the ybc shifts + qb0 drains.
        ybc = singles.tile([128, M], bf16)
        for g in range(NGRP):
            ps_y = psp.tile([128, GRP * KSEG], f32, tag="ps")
            for si in range(GRP):
                s = g * GRP + si
                sq = sqp.tile([128, KSEG], bf16, tag="sq")
                nc.scalar.activation(
                    out=sq, in_=feat2T_sb[:, s * KSEG:(s + 1) * KSEG],
                    func=AF.Square, bias=0.0, scale=1.0)
                nc.tensor.matmul(ps_y[:, si * KSEG:(si + 1) * KSEG],
                                 lhsT=ones_mat, rhs=sq, start=True, stop=True)
            nc.vector.tensor_scalar(
                out=ybc[:, g * GRP * KSEG:(g + 1) * GRP * KSEG],
                in0=ps_y, scalar1=-128.0, scalar2=-0.5,
                op0=ALU.add, op1=ALU.mult)

        # ---- x_sq (+128 shift) for the sqrt bias -- ACT Square with fused
        # row-sum (also in ACT's idle startup window)
        x_sq = singles.tile([128, QB], f32)
        for b in range(QB):
            fsq = smallp.tile([128, D], f32, tag="fsq")
            nc.scalar.activation(out=fsq, in_=featn_sb[:, b, :],
                                 func=AF.Square, bias=0.0, scale=1.0,
                                 accum_out=x_sq[:, b:b + 1])
        xb = singles.tile([128, QB], f32)
        nc.vector.tensor_scalar_add(xb, x_sq, 128.0)

        # ---- picked-label squared distance (DVE, early: ACT needs pdist in
        # the last sqrt-table window)
        psq = singles.tile([128, QB], f32)
        diff_all = singles.tile([128, QB, D], f32)
        nc.vector.tensor_sub(diff_all, featn_sb, sel_sb)
        for b in range(QB):
            dsq = smallp.tile([128, D], f32, tag="fsq")
            nc.vector.tensor_mul(dsq, diff_all[:, b, :], diff_all[:, b, :])
            nc.vector.reduce_sum(psq[:, b:b + 1], dsq, axis=AX.X)

        # ---- main pipeline, emitted in pair-of-qb chunks so every engine's
        # priority queue interleaves: [mains+drains x2qb][sqrt x2][zrow]
        # [exp x2] ... ACT table phases are data-chained in both directions
        # (zrow: exp after last sqrt of the pair; m2: sqrt of pair h after
        # exp of pair h-1) so the ~2.7us table reloads stay at 2 per pair.
        NPH = 4
        PER = QB // NPH
        S = singles.tile([128, QB], f32)
        pdist = singles.tile([128, QB], f32)
        zrows = singles.tile([128, NPH], f32)
        m2s = singles.tile([128, NPH], f32)
        dist_tiles = []
        for h in range(NPH):
            qbs = range(h * PER, (h + 1) * PER)
            for b in qbs:
                dist_t = distp.tile([128, M], bf16, tag="dist")
                dist_tiles.append(dist_t)
                for g in range(NGRP):
                    ps = psp.tile([128, GRP * KSEG], f32, tag="ps")
                    for si in range(GRP):
                        nc.tensor.matmul(
                            ps[:, si * KSEG:(si + 1) * KSEG],
                            lhsT=featT_sb[:, b * 128:(b + 1) * 128],
                            rhs=feat2T_sb[:, (g * GRP + si) * KSEG:
                                          (g * GRP + si + 1) * KSEG],
                            start=True, stop=True)
                    nc.vector.tensor_add(
                        dist_t[:, g * GRP * KSEG:(g + 1) * GRP * KSEG],
                        ps, ybc[:, g * GRP * KSEG:(g + 1) * GRP * KSEG])
            if h == 0:
                scale_h = -2.0
            else:
                nc.vector.tensor_scalar(
                    out=m2s[:, h:h + 1], in0=S[:, h * PER - 1:h * PER],
                    scalar1=0.0, scalar2=-2.0, op0=ALU.mult, op1=ALU.add)
                scale_h = m2s[:, h:h + 1]
            for b in qbs:
                nc.scalar.activation(
                    out=dist_tiles[b], in_=dist_tiles[b], func=AF.Sqrt,
                    bias=xb[:, b:b + 1], scale=scale_h)
            if h == NPH - 1:
                # picked-label distance; still inside a sqrt-table window
                nc.scalar.activation(out=pdist, in_=psq, func=AF.Sqrt,
                                     bias=0.0, scale=1.0)
                nc.vector.tensor_scalar_mul(zrows[:, h:h + 1],
                                            pdist[:, 0:1], 0.0)
            else:
                last = (h + 1) * PER - 1
                nc.vector.tensor_scalar_mul(zrows[:, h:h + 1],
                                            dist_tiles[last][:, M - 1:M], 0.0)
            for b in qbs:
                nc.scalar.activation(
                    out=dist_tiles[b], in_=dist_tiles[b], func=AF.Exp,
                    bias=zrows[:, h:h + 1], scale=neg_temp[:, 0:1],
                    accum_out=S[:, b:b + 1])

        # ---- finals: loss_row = Ln(S) + temp * pdist
        logz = singles.tile([128, QB], f32)
        nc.scalar.activation(out=logz, in_=S, func=AF.Ln, bias=0.0, scale=1.0)
        picked = singles.tile([128, QB], f32)
        nc.vector.tensor_scalar_mul(picked, pdist, pos_temp[:, 0:1])
        loss_t = singles.tile([128, QB], f32)
        nc.vector.tensor_add(loss_t, picked, logz)
        nc.sync.dma_start(out=out_d, in_=loss_t)


def build_program():
    nc = bacc.Bacc("TRN2", target_bir_lowering=False, debug=False,
                   num_devices=N_CORES)
    featT = nc.dram_tensor("featT", [D, NQ], bf16, kind="ExternalInput").ap()
    featn = nc.dram_tensor("featn", [NQ, D], bf16, kind="ExternalInput").ap()
    feat2T = nc.dram_tensor("feat2T", [D, M], bf16, kind="ExternalInput").ap()
    sel = nc.dram_tensor("sel", [NQ, D], bf16, kind="ExternalInput").ap()
    temp = nc.dram_tensor("temp", [1, 1], f32, kind="ExternalInput").ap()
    out = nc.dram_tensor("out", [128, QB], f32, kind="ExternalOutput").ap()
    with tile.TileContext(nc) as tc:
        _body(tc, out, featT, featn, feat2T, sel, temp)
    nc.compile()
    return nc


def make_in_maps(feat, feat2, temp, labels):
    import ml_dtypes
    feat = np.ascontiguousarray(np.asarray(feat, dtype=np.float32))
    feat2 = np.ascontiguousarray(np.asarray(feat2, dtype=np.float32))
    labels_np = np.asarray(labels).astype(np.int64)
    temp_np = np.asarray(temp, dtype=np.float32).reshape(1, 1)
    feat2T = np.ascontiguousarray(feat2.T).astype(ml_dtypes.bfloat16)
    sel_full = feat2[labels_np]
    in_maps = []
    for c in range(N_CORES):
        fs = feat[c * NQ:(c + 1) * NQ]
        in_maps.append({
            "featT": np.ascontiguousarray(fs.T).astype(ml_dtypes.bfloat16),
            "featn": fs.astype(ml_dtypes.bfloat16),
            "feat2T": feat2T,
            "sel": np.ascontiguousarray(sel_full[c * NQ:(c + 1) * NQ]).astype(ml_dtypes.bfloat16),
            "temp": temp_np,
        })
    return in_maps


def combine_outputs(per_core_outs):
    # out[p, b] is the loss for query q = b*128 + p of that core's shard
    rows = [np.asarray(o).T.reshape(-1) for o in per_core_outs]
    return np.float32(np.concatenate(rows).mean())


_PROGRAM = None


def kernel(feat, feat2, temp, labels):
    global _PROGRAM
    if _PROGRAM is None:
        _PROGRAM = build_program()
    in_maps = make_in_maps(feat, feat2, temp, labels)
    res = run_bass_kernel_spmd(_PROGRAM, in_maps, core_ids=list(range(N_CORES)))
    return combine_outputs([r["out"] for r in res.results])
